# revision 10
# baseline (speedup 1.0000x reference)
"""Trainium2 Bass kernel for a 3-layer Lorentz (hyperboloid) MLP.

Math: the reference chains lorentz_linear + inter-layer projx(expmap0(logmap0(.))).
Algebraically, expmap0 -> projx -> logmap0 round-trips cancel: the inter-layer op
on the tangent vector y is exactly "zero the time component, clamp the row norm
of y[1:] to 10".  So the network is:

  t0 = logmap0(x)                       (row scale d/||xs|| on xs, time comp 0)
  y1 = t0 @ W1.T + b1 ; t1 = clamp(y1)  (zero col 0, clamp row norm to 10)
  y2 = t1 @ W2.T + b2 ; t2 = clamp(y2)
  y3 = t2 @ W3.T + b3
  out = [cosh(nc), sinh(nc)/n * y3[1:]] with n=clip(||y3[1:]||,eps), nc=min(n,10)

Layout: everything on-chip is FEATURE-major ([feat, token]); weights are
pre-transposed/blocked and pre-cast to bf16 on the host.  Row-wise norms are
computed with ones-vector matmuls on the TensorEngine (partition-dim reduce),
per-token scales broadcast across partitions with gpsimd.partition_broadcast.

Sharding: pure data-parallel over tokens - 8192 tokens -> 8 cores x 1024.
"""

import os
import sys
import functools

import numpy as np
import ml_dtypes


def _import_concourse():
    try:
        import concourse  # noqa: F401
    except ImportError:
        for p in ("/opt/trn_rl_repo", "/root/.axon_site/_ro/trn_rl_repo"):
            if os.path.isdir(p) and p not in sys.path:
                sys.path.insert(0, p)
        import concourse  # noqa: F401


_import_concourse()

import concourse.bass as bass  # noqa: E402,F401
import concourse.bacc as bacc  # noqa: E402
import concourse.mybir as mybir  # noqa: E402
import concourse.tile as tile  # noqa: E402
from concourse import bass_utils  # noqa: E402

F32 = mybir.dt.float32
BF16 = mybir.dt.bfloat16
AF = mybir.ActivationFunctionType
ALU = mybir.AluOpType

P = 128
N_CORES = 8
EPS = 1e-7
MAX_TAN_NORM = 10.0

# Full-problem dims (hardcoded per spec)
TOK, D_IN, D_HID, D_OUT = 8192, 1024, 4096, 1024
TOKPC = TOK // N_CORES  # tokens per core


def build_nc(tokpc=TOKPC, din=D_IN, dhid=D_HID, dout=D_OUT, ch=512):
    """Build + compile the per-core Bass program."""
    assert tokpc % ch == 0
    nch = tokpc // ch
    kt1, mt1 = din // P, dhid // P
    kt2, mt2 = dhid // P, dhid // P
    kt3, mt3 = dhid // P, dout // P

    nc = bacc.Bacc("TRN2", target_bir_lowering=False, debug=False,
                   num_devices=N_CORES)

    xt_d = nc.dram_tensor("xt", [din, tokpc], F32, kind="ExternalInput")
    w1_d = nc.dram_tensor("w1", [mt1 * P, din], BF16, kind="ExternalInput")
    w2_d = nc.dram_tensor("w2", [mt2 * P, dhid], BF16, kind="ExternalInput")
    w3_d = nc.dram_tensor("w3", [mt3 * P, dhid], BF16, kind="ExternalInput")
    b1_d = nc.dram_tensor("b1", [P, mt1], F32, kind="ExternalInput")
    b2_d = nc.dram_tensor("b2", [P, mt2], F32, kind="ExternalInput")
    b3_d = nc.dram_tensor("b3", [P, mt3], F32, kind="ExternalInput")
    out_d = nc.dram_tensor("out", [dout, tokpc], F32, kind="ExternalOutput")

    with tile.TileContext(nc) as tc:
        _build_tile_program(tc, nc, dict(
            tokpc=tokpc, din=din, dhid=dhid, dout=dout, ch=ch, nch=nch,
            kt1=kt1, mt1=mt1, kt2=kt2, mt2=mt2, kt3=kt3, mt3=mt3,
            xt=xt_d, w1=w1_d, w2=w2_d, w3=w3_d,
            b1=b1_d, b2=b2_d, b3=b3_d, out=out_d,
        ))
    nc.compile()
    return nc


def _build_tile_program(tc, nc, C):
    tokpc, ch, nch = C["tokpc"], C["ch"], C["nch"]

    # long-lived pools (live for the whole program)
    const = tc.alloc_tile_pool(name="const", bufs=1)
    scalL = tc.alloc_tile_pool(name="scalL", bufs=5)   # [1, tokpc] f32
    scalS = tc.alloc_tile_pool(name="scalS", bufs=7)   # [1, ch] f32
    bcast = tc.alloc_tile_pool(name="bcast", bufs=2)
    sqp = tc.alloc_tile_pool(name="sq", bufs=4)
    wp = tc.alloc_tile_pool(name="wt", bufs=3)
    psy = tc.alloc_tile_pool(name="psy", bufs=4, space="PSUM")
    psn = tc.alloc_tile_pool(name="psn", bufs=2, space="PSUM")
    outp = tc.alloc_tile_pool(name="outp", bufs=2)

    ones_k = const.tile([P, 1], BF16, tag="ones_k")
    nc.vector.memset(ones_k[:], 1.0)
    bias1 = const.tile([P, C["mt1"]], F32, tag="bias1")
    nc.sync.dma_start(bias1[:], C["b1"].ap())
    bias2 = const.tile([P, C["mt2"]], F32, tag="bias2")
    nc.sync.dma_start(bias2[:], C["b2"].ap())
    bias3 = const.tile([P, C["mt3"]], F32, tag="bias3")
    nc.sync.dma_start(bias3[:], C["b3"].ap())

    def stile_l():
        return scalL.tile([1, tokpc], F32, tag="sl", name="sl")

    def stile_s():
        return scalS.tile([1, ch], F32, tag="ss", name="ss")

    def norm_accum_tiles():
        """PSUM accumulators for per-token sum of squares, one per chunk."""
        return [psn.tile([1, ch], F32, tag="psn", name=f"psn{_}")
                for _ in range(nch)]

    def bcast_scale(s_full):
        """[1, tokpc] f32 -> [P, tokpc] f32 broadcast."""
        sb = bcast.tile([P, tokpc], F32, tag="sb", name="sb")
        nc.gpsimd.partition_broadcast(sb[:], s_full[:])
        return sb

    # ---------------- Phase 0: load x, logmap0 -> t0 (bf16) ----------------
    xfp = tc.alloc_tile_pool(name="xf", bufs=1, side="left")
    t0p = tc.alloc_tile_pool(name="t0", bufs=1, side="right")

    xf = []
    for k in range(C["kt1"]):
        t = xfp.tile([P, tokpc], F32, tag=f"xf{k}", name=f"xf{k}")
        nc.sync.dma_start(t[:], C["xt"].ap()[k * P:(k + 1) * P, :])
        xf.append(t)
    ps_n0 = norm_accum_tiles()
    for k in range(C["kt1"]):
        for c in range(nch):
            sq = sqp.tile([P, ch], BF16, tag="sq", name="sq")
            nc.vector.tensor_tensor(sq[:], xf[k][:, c * ch:(c + 1) * ch],
                                    xf[k][:, c * ch:(c + 1) * ch], ALU.mult)
            if k == 0:
                nc.vector.memset(sq[0:1, :], 0.0)  # exclude time component
            nc.tensor.matmul(ps_n0[c][:], ones_k[:], sq[:],
                             start=(k == 0), stop=(k == C["kt1"] - 1))
    # d = arccosh(max(x0, 1+eps)) = ln(xc + sqrt(xc^2 - 1))
    xc = stile_l()
    nc.vector.tensor_scalar_max(xc[:], xf[0][0:1, :], 1.0 + EPS)
    t2_ = stile_l()
    nc.vector.tensor_tensor(t2_[:], xc[:], xc[:], ALU.mult)
    nc.vector.tensor_scalar_add(t2_[:], t2_[:], -1.0)
    r_ = stile_l()
    nc.scalar.activation(r_[:], t2_[:], AF.Sqrt)
    nc.vector.tensor_tensor(r_[:], xc[:], r_[:], ALU.add)
    d_ = stile_l()
    nc.scalar.activation(d_[:], r_[:], AF.Ln)
    # ns = max(sqrt(ssq), eps); s0 = d / ns
    s0 = stile_l()
    for c in range(nch):
        n_ = stile_s()
        nc.scalar.activation(n_[:], ps_n0[c][:], AF.Sqrt)
        nc.vector.tensor_scalar_max(n_[:], n_[:], EPS)
        rec = stile_s()
        nc.vector.reciprocal(rec[:], n_[:])
        nc.vector.tensor_tensor(s0[:, c * ch:(c + 1) * ch],
                                d_[:, c * ch:(c + 1) * ch], rec[:], ALU.mult)
    s0b = bcast_scale(s0)
    t0 = []
    for k in range(C["kt1"]):
        t = t0p.tile([P, tokpc], BF16, tag=f"t0_{k}", name=f"t0_{k}")
        nc.vector.tensor_tensor(t[:], xf[k][:], s0b[:], ALU.mult)
        t0.append(t)
    nc.vector.memset(t0[0][0:1, :], 0.0)
    xfp.release()

    # ---------------- GEMM layers ----------------
    def gemm_layer(tin, w_d, bias_t, kt, mt, out_pool, out_dtype, tag):
        """y[m] = sum_k w[k,m].T @ tin[k]; evict(+bias); zero row0 of m=0;
        accumulate row sum-of-squares (norm matmuls pipelined one m behind
        so the PE never stalls on the ACT epilogue).  Returns (tiles, ps_norm)."""
        ps_norm = norm_accum_tiles()
        tout = []
        pending = None  # (sq tiles, m) whose norm-MMs are deferred

        def flush_pending(last):
            nonlocal pending
            if pending is None:
                return
            psq, pm = pending
            for c in range(nch):
                nc.tensor.matmul(ps_norm[c][:], ones_k[:], psq[c][:],
                                 start=(pm == 0), stop=last)
            pending = None

        for m in range(mt):
            wm = wp.tile([P, kt * P], BF16, tag="wtile", name="wm")
            nc.sync.dma_start(wm[:], w_d.ap()[m * P:(m + 1) * P, :])
            pss = [psy.tile([P, ch], F32, tag="psy", name=f"psy{_}")
                   for _ in range(nch)]
            for k in range(kt):
                for c in range(nch):
                    nc.tensor.matmul(pss[c][:], wm[:, k * P:(k + 1) * P],
                                     tin[k][:, c * ch:(c + 1) * ch],
                                     start=(k == 0), stop=(k == kt - 1))
            flush_pending(last=False)
            ty = out_pool.tile([P, tokpc], out_dtype, tag=f"{tag}{m}",
                               name=f"{tag}{m}")
            sqs = []
            for c in range(nch):
                nc.scalar.activation(ty[:, c * ch:(c + 1) * ch], pss[c][:],
                                     AF.Identity, bias=bias_t[:, m:m + 1],
                                     scale=1.0)
                sq = sqp.tile([P, ch], BF16, tag="sq", name="sq")
                nc.scalar.activation(sq[:], pss[c][:], AF.Square,
                                     bias=bias_t[:, m:m + 1], scale=1.0)
                if m == 0:
                    nc.vector.memset(sq[0:1, :], 0.0)
                sqs.append(sq)
            if m == 0:
                nc.vector.memset(ty[0:1, :], 0.0)
            pending = (sqs, m)
            tout.append(ty)
        flush_pending(last=True)
        return tout, ps_norm

    def clamp_scale(ps_norm):
        """s = min(max(sqrt(ssq),eps),10) / max(sqrt(ssq),eps) -> [1,tokpc]."""
        s = stile_l()
        for c in range(nch):
            n_ = stile_s()
            nc.scalar.activation(n_[:], ps_norm[c][:], AF.Sqrt)
            nc.vector.tensor_scalar_max(n_[:], n_[:], EPS)
            num = stile_s()
            nc.vector.tensor_scalar_min(num[:], n_[:], MAX_TAN_NORM)
            rec = stile_s()
            nc.vector.reciprocal(rec[:], n_[:])
            nc.vector.tensor_tensor(s[:, c * ch:(c + 1) * ch], num[:], rec[:],
                                    ALU.mult)
        return s

    def apply_scale(tiles, sb):
        for t in tiles:
            nc.vector.tensor_tensor(t[:], t[:], sb[:], ALU.mult)

    t1p = tc.alloc_tile_pool(name="t1", bufs=1, side="left")
    t1, psn1 = gemm_layer(t0, C["w1"], bias1, C["kt1"], C["mt1"],
                          t1p, BF16, "t1_")
    apply_scale(t1, bcast_scale(clamp_scale(psn1)))
    t0p.release()

    t2p = tc.alloc_tile_pool(name="t2", bufs=1, side="right")
    t2, psn2 = gemm_layer(t1, C["w2"], bias2, C["kt2"], C["mt2"],
                          t2p, BF16, "t2_")
    apply_scale(t2, bcast_scale(clamp_scale(psn2)))
    t1p.release()

    y3p = tc.alloc_tile_pool(name="y3", bufs=1, side="left")
    y3, psn3 = gemm_layer(t2, C["w3"], bias3, C["kt3"], C["mt3"],
                          y3p, F32, "y3_")
    t2p.release()

    # final: n=max(sqrt,eps); ncl=min(n,10); s3=sinh(ncl)/n; x0=cosh(ncl)
    s3 = stile_l()
    cosh_t = stile_l()
    for c in range(nch):
        n_ = stile_s()
        nc.scalar.activation(n_[:], psn3[c][:], AF.Sqrt)
        nc.vector.tensor_scalar_max(n_[:], n_[:], EPS)
        ncl = stile_s()
        nc.vector.tensor_scalar_min(ncl[:], n_[:], MAX_TAN_NORM)
        e_ = stile_s()
        nc.scalar.activation(e_[:], ncl[:], AF.Exp)
        nn = stile_s()
        nc.vector.tensor_scalar_mul(nn[:], ncl[:], -1.0)
        en = stile_s()
        nc.scalar.activation(en[:], nn[:], AF.Exp)
        sh = stile_s()
        nc.vector.tensor_tensor(sh[:], e_[:], en[:], ALU.subtract)
        nc.vector.tensor_scalar_mul(sh[:], sh[:], 0.5)
        co = cosh_t[:, c * ch:(c + 1) * ch]
        nc.vector.tensor_tensor(co, e_[:], en[:], ALU.add)
        nc.vector.tensor_scalar_mul(co, co, 0.5)
        rec = stile_s()
        nc.vector.reciprocal(rec[:], n_[:])
        nc.vector.tensor_tensor(s3[:, c * ch:(c + 1) * ch], sh[:], rec[:],
                                ALU.mult)
    s3b = bcast_scale(s3)
    for m in range(C["mt3"]):
        ot = outp.tile([P, tokpc], F32, tag="ot", name="ot")
        nc.vector.tensor_tensor(ot[:], y3[m][:], s3b[:], ALU.mult)
        if m == 0:
            nc.vector.tensor_copy(ot[0:1, :], cosh_t[:])
        nc.sync.dma_start(C["out"].ap()[m * P:(m + 1) * P, :], ot[:])
    y3p.release()
    for p in (outp, psn, psy, wp, sqp, bcast, scalS, scalL, const):
        p.release()


# ---------------- host-side prep + entry point ----------------

def _block_weight(w):
    """W [dout, din] f32 -> [mt*128, din] bf16 with row m*128+p holding, for
    each k-tile, lhsT tile (k,m) row p: out[m*128+p, k*128+j] = W.T[k*128+p,
    m*128+j].  One fully-contiguous [128, kt*128] DMA per m-tile."""
    dout, din = w.shape
    mt, kt = dout // P, din // P
    w = np.asarray(w, dtype=np.float32)
    blocked = (w.reshape(mt, P, kt, P)       # [m, j, k, p]
                .transpose(0, 3, 2, 1)       # [m, p, k, j]
                .reshape(mt * P, din))
    return np.ascontiguousarray(blocked.astype(ml_dtypes.bfloat16))


def _prep_bias(b, mt):
    """b [d] -> [128, mt] f32 with out[p, m] = b[m*128+p]."""
    return np.ascontiguousarray(
        np.asarray(b, dtype=np.float32).reshape(mt, P).T)


@functools.lru_cache(maxsize=1)
def _get_nc():
    return build_nc()


def prep_in_maps(x_hyp, W1, b1, W2, b2, W3, b3):
    w1b = _block_weight(W1)
    w2b = _block_weight(W2)
    w3b = _block_weight(W3)
    b1c = _prep_bias(b1, D_HID // P)
    b2c = _prep_bias(b2, D_HID // P)
    b3c = _prep_bias(b3, D_OUT // P)
    x = np.asarray(x_hyp, dtype=np.float32)
    in_maps = []
    for c in range(N_CORES):
        shard = x[c * TOKPC:(c + 1) * TOKPC, :]  # [tokpc, din]
        xt = np.ascontiguousarray(shard.T)  # [din, tokpc]
        in_maps.append(dict(xt=xt, w1=w1b, w2=w2b, w3=w3b,
                            b1=b1c, b2=b2c, b3=b3c))
    return in_maps


def kernel(x_hyp, W1, b1, W2, b2, W3, b3):
    nc = _get_nc()
    in_maps = prep_in_maps(x_hyp, W1, b1, W2, b2, W3, b3)
    res = bass_utils.run_bass_kernel_spmd(nc, in_maps,
                                          core_ids=list(range(N_CORES)))
    parts = [np.asarray(res.results[c]["out"]).T for c in range(N_CORES)]
    return np.ascontiguousarray(np.concatenate(parts, axis=0),
                                dtype=np.float32)


# revision 17
# speedup vs baseline: 3360.6261x; 3360.6261x over previous
"""Trainium2 Bass kernel for a 3-layer Lorentz (hyperboloid) MLP.

Math: the reference chains lorentz_linear + inter-layer projx(expmap0(logmap0(.))).
Algebraically, expmap0 -> projx -> logmap0 round-trips cancel: the inter-layer op
on the tangent vector y is exactly "zero the time component, clamp the row norm
of y[1:] to 10".  So the network is:

  t0 = logmap0(x)                       (row scale d/||xs|| on xs, time comp 0)
  y1 = t0 @ W1.T + b1 ; t1 = clamp(y1)  (zero col 0, clamp row norm to 10)
  y2 = t1 @ W2.T + b2 ; t2 = clamp(y2)
  y3 = t2 @ W3.T + b3
  out = [cosh(nc), sinh(nc)/n * y3[1:]] with n=clip(||y3[1:]||,eps), nc=min(n,10)

With zero biases (the shipped case), per-token scales commute through the
GEMMs, so all clamp/logmap scales are folded into one cumulative per-token
scale applied once at the very end ("fold" mode) - the PE runs the three
GEMMs back to back with no inter-layer barrier.  With nonzero biases a
general barrier path (scale applied between layers) is built instead.

Layout: everything on-chip is FEATURE-major ([feat, token]); weights are
pre-transposed/blocked/bf16-cast on the host so each m-tile loads with one
fully contiguous DMA.  Row-wise (per-token) norms are ones-vector matmuls on
the TensorEngine (partition-dim reduction), pipelined one m-tile behind the
main GEMM stream; per-token scales broadcast across partitions with
gpsimd.partition_broadcast.

Sharding: pure data-parallel over tokens - 8192 tokens -> 8 cores x 1024.
"""

import os
import sys
import functools

import numpy as np
import ml_dtypes


def _import_concourse():
    try:
        import concourse  # noqa: F401
    except ImportError:
        for p in ("/opt/trn_rl_repo", "/root/.axon_site/_ro/trn_rl_repo"):
            if os.path.isdir(p) and p not in sys.path:
                sys.path.insert(0, p)
        import concourse  # noqa: F401


_import_concourse()

import concourse.bass as bass  # noqa: E402,F401
import concourse.bacc as bacc  # noqa: E402
import concourse.mybir as mybir  # noqa: E402
import concourse.tile as tile  # noqa: E402
from concourse import bass_utils  # noqa: E402

F32 = mybir.dt.float32
BF16 = mybir.dt.bfloat16
AF = mybir.ActivationFunctionType
ALU = mybir.AluOpType

P = 128
N_CORES = 8
EPS = 1e-7
MAX_TAN_NORM = 10.0

# Full-problem dims (hardcoded per spec)
TOK, D_IN, D_HID, D_OUT = 8192, 1024, 4096, 1024
TOKPC = TOK // N_CORES  # tokens per core


def build_nc(tokpc=TOKPC, din=D_IN, dhid=D_HID, dout=D_OUT, ch=512,
             repeat=1, fold=False):
    """Build + compile the per-core Bass program."""
    assert tokpc % ch == 0
    nch = tokpc // ch
    kt1, mt1 = din // P, dhid // P
    kt2, mt2 = dhid // P, dhid // P
    kt3, mt3 = dhid // P, dout // P

    nc = bacc.Bacc("TRN2", target_bir_lowering=False, debug=False,
                   num_devices=N_CORES)

    xt_d = nc.dram_tensor("xt", [din, tokpc], BF16, kind="ExternalInput")
    x0_d = nc.dram_tensor("x0", [1, tokpc], F32, kind="ExternalInput")
    w1_d = nc.dram_tensor("w1", [mt1 * P, din], BF16, kind="ExternalInput")
    w2_d = nc.dram_tensor("w2", [mt2 * P, dhid], BF16, kind="ExternalInput")
    w3_d = nc.dram_tensor("w3", [mt3 * P, dhid], BF16, kind="ExternalInput")
    b1_d = nc.dram_tensor("b1", [P, mt1], F32, kind="ExternalInput")
    b2_d = nc.dram_tensor("b2", [P, mt2], F32, kind="ExternalInput")
    b3_d = nc.dram_tensor("b3", [P, mt3], F32, kind="ExternalInput")
    out_d = nc.dram_tensor("out", [dout, tokpc], F32, kind="ExternalOutput")

    with tile.TileContext(nc) as tc:
        _build_tile_program(tc, nc, dict(
            tokpc=tokpc, din=din, dhid=dhid, dout=dout, ch=ch, nch=nch,
            kt1=kt1, mt1=mt1, kt2=kt2, mt2=mt2, kt3=kt3, mt3=mt3,
            xt=xt_d, x0=x0_d, w1=w1_d, w2=w2_d, w3=w3_d,
            b1=b1_d, b2=b2_d, b3=b3_d, out=out_d,
        ), repeat=repeat, fold=fold)
    nc.compile()
    return nc


def _build_tile_program(tc, nc, C, repeat=1, fold=False):
    tokpc, ch, nch = C["tokpc"], C["ch"], C["nch"]

    # long-lived pools
    const = tc.alloc_tile_pool(name="const", bufs=1)
    scalL = tc.alloc_tile_pool(name="scalL", bufs=5)   # [1, tokpc] f32
    scalS = tc.alloc_tile_pool(name="scalS", bufs=6)   # [1, ch] f32
    bcast = tc.alloc_tile_pool(name="bcast", bufs=1 if fold else 2)
    sqp = tc.alloc_tile_pool(name="sq", bufs=2)
    accp = tc.alloc_tile_pool(name="acc", bufs=4)
    wp = tc.alloc_tile_pool(name="wt", bufs=3)
    psy = tc.alloc_tile_pool(name="psy", bufs=4, space="PSUM")
    psn = tc.alloc_tile_pool(name="psn", bufs=4, space="PSUM")
    outp = tc.alloc_tile_pool(name="outp", bufs=4)

    ones_k = const.tile([P, 1], BF16, tag="ones_k")
    nc.vector.memset(ones_k[:], 1.0)
    bias1 = const.tile([P, C["mt1"]], F32, tag="bias1")
    nc.sync.dma_start(bias1[:], C["b1"].ap())
    bias2 = const.tile([P, C["mt2"]], F32, tag="bias2")
    nc.sync.dma_start(bias2[:], C["b2"].ap())
    bias3 = const.tile([P, C["mt3"]], F32, tag="bias3")
    nc.sync.dma_start(bias3[:], C["b3"].ap())

    def stile_l():
        return scalL.tile([1, tokpc], F32, tag="sl", name="sl")

    def stile_s():
        return scalS.tile([1, ch], F32, tag="ss", name="ss")

    def norm_accum_tiles():
        return [psn.tile([1, ch], F32, tag="psn", name=f"psn{_}")
                for _ in range(nch)]

    def bcast_full(s_full):
        sb = bcast.tile([P, tokpc], F32, tag="sb", name="sb")
        nc.gpsimd.partition_broadcast(sb[:], s_full[:])
        return sb

    # ---------------- GEMM layer (layers 1, 2) ----------------
    ones_f = const.tile([P, 1], F32, tag="ones_f", name="ones_f")
    nc.vector.memset(ones_f[:], 1.0)

    def gemm_layer(tin, w_d, bias_t, kt, mt, out_pool, out_dtype, tag):
        """y[m] = sum_k w[k,m].T @ tin[k]; ACT evicts (+bias) and squares
        straight from PSUM; squares accumulate on the idle DVE (f32) and a
        single fp32 ones-matmul per chunk does the final partition-reduce."""
        ps_norm = norm_accum_tiles()
        accs = [accp.tile([P, ch], F32, tag="acc", name=f"acc{_}")
                for _ in range(nch)]
        tout = []
        for m in range(mt):
            wm = wp.tile([P, kt * P], BF16, tag="wtile", name="wm")
            nc.sync.dma_start(wm[:], w_d.ap()[m * P:(m + 1) * P, :])
            pss = [psy.tile([P, ch], F32, tag="psy", name=f"psy{_}")
                   for _ in range(nch)]
            for k in range(kt):
                for c in range(nch):
                    nc.tensor.matmul(pss[c][:], wm[:, k * P:(k + 1) * P],
                                     tin[k][:, c * ch:(c + 1) * ch],
                                     start=(k == 0), stop=(k == kt - 1))
            ty = out_pool.tile([P, tokpc], out_dtype, tag=f"{tag}{m}",
                               name=f"{tag}{m}")
            for c in range(nch):
                nc.scalar.activation(ty[:, c * ch:(c + 1) * ch], pss[c][:],
                                     AF.Identity, bias=bias_t[:, m:m + 1],
                                     scale=1.0)
                if m == 0:
                    nc.scalar.activation(accs[c][:], pss[c][:], AF.Square,
                                         bias=bias_t[:, m:m + 1], scale=1.0)
                    nc.vector.memset(accs[c][0:1, :], 0.0)
                else:
                    sq = sqp.tile([P, ch], F32, tag="sq", name="sq")
                    nc.scalar.activation(sq[:], pss[c][:], AF.Square,
                                         bias=bias_t[:, m:m + 1], scale=1.0)
                    nc.vector.tensor_tensor(accs[c][:], accs[c][:], sq[:],
                                            ALU.add)
            if m == 0:
                nc.vector.memset(ty[0:1, :], 0.0)
            tout.append(ty)
        for c in range(nch):
            nc.tensor.matmul(ps_norm[c][:], ones_f[:], accs[c][:],
                             start=True, stop=True)
        return tout, ps_norm

    def clamp_scale(ps_norm):
        """Barrier path: s = min(max(sqrt(ssq),eps),10)/max(sqrt(ssq),eps)."""
        s = stile_l()
        for c in range(nch):
            n_ = stile_s()
            nc.scalar.activation(n_[:], ps_norm[c][:], AF.Sqrt)
            nc.vector.tensor_scalar_max(n_[:], n_[:], EPS)
            num = stile_s()
            nc.vector.tensor_scalar_min(num[:], n_[:], MAX_TAN_NORM)
            rec = stile_s()
            nc.vector.reciprocal(rec[:], n_[:])
            nc.vector.tensor_tensor(s[:, c * ch:(c + 1) * ch], num[:], rec[:],
                                    ALU.mult)
        return s

    def apply_scale(tiles, sb):
        for t in tiles:
            nc.vector.tensor_tensor(t[:], t[:], sb[:], ALU.mult)

    def clamp_chain(ps_norm, cs_prev):
        """Fold path: true norm n = max(cs_prev*sqrt(ssq_raw), eps);
        cs_new = cs_prev*min(n,10)/n.  [1,tokpc] ops only - nothing on the
        GEMM critical path waits on this."""
        cs_new = stile_l()
        for c in range(nch):
            sl = slice(c * ch, (c + 1) * ch)
            n_ = stile_s()
            nc.scalar.activation(n_[:], ps_norm[c][:], AF.Sqrt)
            nc.vector.tensor_tensor(n_[:], n_[:], cs_prev[:, sl], ALU.mult)
            nc.vector.tensor_scalar_max(n_[:], n_[:], EPS)
            num = stile_s()
            nc.vector.tensor_scalar_min(num[:], n_[:], MAX_TAN_NORM)
            rec = stile_s()
            nc.vector.reciprocal(rec[:], n_[:])
            nc.vector.tensor_tensor(rec[:], num[:], rec[:], ALU.mult)
            nc.vector.tensor_tensor(cs_new[:, sl], cs_prev[:, sl], rec[:],
                                    ALU.mult)
        return cs_new

    def body():
        # ---------- Phase 0: load bf16 xs (= raw t0), input norm, s0 ----------
        t0p = tc.alloc_tile_pool(name="t0", bufs=1, side="right")
        t0 = []
        for k in range(C["kt1"]):
            t = t0p.tile([P, tokpc], BF16, tag=f"t0_{k}", name=f"t0_{k}")
            nc.sync.dma_start(t[:], C["xt"].ap()[k * P:(k + 1) * P, :])
            t0.append(t)
        nc.vector.memset(t0[0][0:1, :], 0.0)  # zero time component
        x0t = stile_l()
        nc.sync.dma_start(x0t[:], C["x0"].ap())

        ps_n0 = norm_accum_tiles()
        acc0 = [accp.tile([P, ch], F32, tag="acc", name=f"acc0_{_}")
                for _ in range(nch)]
        for k in range(C["kt1"]):
            for c in range(nch):
                if k == 0:
                    nc.scalar.activation(acc0[c][:],
                                         t0[k][:, c * ch:(c + 1) * ch],
                                         AF.Square)
                else:
                    sq = sqp.tile([P, ch], F32, tag="sq", name="sq")
                    nc.scalar.activation(sq[:], t0[k][:, c * ch:(c + 1) * ch],
                                         AF.Square)
                    nc.vector.tensor_tensor(acc0[c][:], acc0[c][:], sq[:],
                                            ALU.add)
        for c in range(nch):
            nc.tensor.matmul(ps_n0[c][:], ones_f[:], acc0[c][:],
                             start=True, stop=True)
        # d = arccosh(max(x0, 1+eps)) = ln(xc + sqrt(xc^2 - 1))
        xc = stile_l()
        nc.vector.tensor_scalar_max(xc[:], x0t[:], 1.0 + EPS)
        t2_ = stile_l()
        nc.vector.tensor_tensor(t2_[:], xc[:], xc[:], ALU.mult)
        nc.vector.tensor_scalar_add(t2_[:], t2_[:], -1.0)
        r_ = stile_l()
        nc.scalar.activation(r_[:], t2_[:], AF.Sqrt)
        nc.vector.tensor_tensor(r_[:], xc[:], r_[:], ALU.add)
        d_ = stile_l()
        nc.scalar.activation(d_[:], r_[:], AF.Ln)
        # s0 = d / max(sqrt(ssq), eps)
        s0 = stile_l()
        for c in range(nch):
            n_ = stile_s()
            nc.scalar.activation(n_[:], ps_n0[c][:], AF.Sqrt)
            nc.vector.tensor_scalar_max(n_[:], n_[:], EPS)
            rec = stile_s()
            nc.vector.reciprocal(rec[:], n_[:])
            nc.vector.tensor_tensor(s0[:, c * ch:(c + 1) * ch],
                                    d_[:, c * ch:(c + 1) * ch], rec[:],
                                    ALU.mult)
        if not fold:
            # scale t0 in place (squares above read pre-scale values; Tile's
            # WAR deps order the in-place multiply after them)
            s0b = bcast_full(s0)
            for k in range(C["kt1"]):
                nc.vector.tensor_tensor(t0[k][:], t0[k][:], s0b[:], ALU.mult)

        # ---------- Layers 1, 2 ----------
        t1p = tc.alloc_tile_pool(name="t1", bufs=1, side="left")
        t1, psn1 = gemm_layer(t0, C["w1"], bias1, C["kt1"], C["mt1"],
                              t1p, BF16, "t1_")
        if fold:
            cs = clamp_chain(psn1, s0)
        else:
            apply_scale(t1, bcast_full(clamp_scale(psn1)))
        t0p.release()

        t2p = tc.alloc_tile_pool(name="t2", bufs=1, side="right")
        t2, psn2 = gemm_layer(t1, C["w2"], bias2, C["kt2"], C["mt2"],
                              t2p, BF16, "t2_")
        if fold:
            cs = clamp_chain(psn2, cs)
        else:
            apply_scale(t2, bcast_full(clamp_scale(psn2)))
        t1p.release()

        # ---------- Layer 3 + expmap0/projx, chunk-split so chunk 0's tail
        # overlaps chunk 1's matmuls ----------
        kt, mt = C["kt3"], C["mt3"]
        y3p = tc.alloc_tile_pool(name="y3", bufs=1, side="left")
        y3 = [y3p.tile([P, tokpc], F32, tag=f"y3_{m}", name=f"y3_{m}")
              for m in range(mt)]
        for c in range(nch):
            sl = slice(c * ch, (c + 1) * ch)
            ps_norm = psn.tile([1, ch], F32, tag="psn", name=f"psn3_{c}")
            acc3 = accp.tile([P, ch], F32, tag="acc", name=f"acc3_{c}")
            for m in range(mt):
                wm = wp.tile([P, kt * P], BF16, tag="wtile", name="wm")
                nc.sync.dma_start(wm[:], C["w3"].ap()[m * P:(m + 1) * P, :])
                ps = psy.tile([P, ch], F32, tag="psy", name="psy3")
                for k in range(kt):
                    nc.tensor.matmul(ps[:], wm[:, k * P:(k + 1) * P],
                                     t2[k][:, sl],
                                     start=(k == 0), stop=(k == kt - 1))
                nc.scalar.activation(y3[m][:, sl], ps[:], AF.Identity,
                                     bias=bias3[:, m:m + 1], scale=1.0)
                if m == 0:
                    nc.scalar.activation(acc3[:], ps[:], AF.Square,
                                         bias=bias3[:, m:m + 1], scale=1.0)
                    nc.vector.memset(acc3[0:1, :], 0.0)
                else:
                    sq = sqp.tile([P, ch], F32, tag="sq", name="sq")
                    nc.scalar.activation(sq[:], ps[:], AF.Square,
                                         bias=bias3[:, m:m + 1], scale=1.0)
                    nc.vector.tensor_tensor(acc3[:], acc3[:], sq[:], ALU.add)
            nc.tensor.matmul(ps_norm[:], ones_f[:], acc3[:],
                             start=True, stop=True)

            # n=max(.,eps); ncl=min(n,10); s3=[cs*]sinh(ncl)/n; x0=cosh(ncl)
            n_ = stile_s()
            nc.scalar.activation(n_[:], ps_norm[:], AF.Sqrt)
            if fold:
                nc.vector.tensor_tensor(n_[:], n_[:], cs[:, sl], ALU.mult)
            nc.vector.tensor_scalar_max(n_[:], n_[:], EPS)
            ncl = stile_s()
            nc.vector.tensor_scalar_min(ncl[:], n_[:], MAX_TAN_NORM)
            e_ = stile_s()
            nc.scalar.activation(e_[:], ncl[:], AF.Exp)
            nn = stile_s()
            nc.vector.tensor_scalar_mul(nn[:], ncl[:], -1.0)
            en = stile_s()
            nc.scalar.activation(en[:], nn[:], AF.Exp)
            sh = stile_s()
            nc.vector.tensor_tensor(sh[:], e_[:], en[:], ALU.subtract)
            nc.vector.tensor_scalar_mul(sh[:], sh[:], 0.5)
            cosh_c = stile_s()
            nc.vector.tensor_tensor(cosh_c[:], e_[:], en[:], ALU.add)
            nc.vector.tensor_scalar_mul(cosh_c[:], cosh_c[:], 0.5)
            rec = stile_s()
            nc.vector.reciprocal(rec[:], n_[:])
            s3 = stile_s()
            nc.vector.tensor_tensor(s3[:], sh[:], rec[:], ALU.mult)
            if fold:
                nc.vector.tensor_tensor(s3[:], s3[:], cs[:, sl], ALU.mult)
            s3b = bcast.tile([P, ch], F32, tag="s3b", name="s3b")
            nc.gpsimd.partition_broadcast(s3b[:], s3[:])
            for m in range(mt):
                ot = outp.tile([P, ch], F32, tag="ot", name="ot")
                eng = nc.vector if m % 2 == 0 else nc.gpsimd
                eng.tensor_tensor(ot[:], y3[m][:, sl], s3b[:], ALU.mult)
                if m == 0:
                    nc.vector.tensor_copy(ot[0:1, :], cosh_c[:])
                nc.sync.dma_start(C["out"].ap()[m * P:(m + 1) * P, sl], ot[:])
        t2p.release()
        y3p.release()

    for _rep in range(repeat):
        body()

    for p in (outp, psn, psy, wp, accp, sqp, bcast, scalS, scalL, const):
        p.release()


# ---------------- host-side prep + entry point ----------------

def _block_weight(w):
    """W [dout, din] f32 -> [mt*128, din] bf16 with row m*128+p holding, for
    each k-tile, lhsT tile (k,m) row p: out[m*128+p, k*128+j] = W.T[k*128+p,
    m*128+j].  One fully-contiguous [128, kt*128] DMA per m-tile."""
    dout, din = w.shape
    mt, kt = dout // P, din // P
    w = np.asarray(w, dtype=np.float32)
    blocked = (w.reshape(mt, P, kt, P)       # [m, j, k, p]
                .transpose(0, 3, 2, 1)       # [m, p, k, j]
                .reshape(mt * P, din))
    return np.ascontiguousarray(blocked.astype(ml_dtypes.bfloat16))


def _prep_bias(b, mt):
    """b [d] -> [128, mt] f32 with out[p, m] = b[m*128+p]."""
    return np.ascontiguousarray(
        np.asarray(b, dtype=np.float32).reshape(mt, P).T)


@functools.lru_cache(maxsize=2)
def _get_nc(fold=False):
    return build_nc(fold=fold)


def prep_in_maps(x_hyp, W1, b1, W2, b2, W3, b3):
    w1b = _block_weight(W1)
    w2b = _block_weight(W2)
    w3b = _block_weight(W3)
    b1c = _prep_bias(b1, D_HID // P)
    b2c = _prep_bias(b2, D_HID // P)
    b3c = _prep_bias(b3, D_OUT // P)
    x = np.asarray(x_hyp, dtype=np.float32)
    in_maps = []
    for c in range(N_CORES):
        shard = x[c * TOKPC:(c + 1) * TOKPC, :]  # [tokpc, din]
        xt = np.ascontiguousarray(shard.T.astype(ml_dtypes.bfloat16))
        x0 = np.ascontiguousarray(shard[:, 0:1].T)  # [1, tokpc] f32
        in_maps.append(dict(xt=xt, x0=x0, w1=w1b, w2=w2b, w3=w3b,
                            b1=b1c, b2=b2c, b3=b3c))
    return in_maps


def kernel(x_hyp, W1, b1, W2, b2, W3, b3):
    fold = not (np.any(b1) or np.any(b2) or np.any(b3))
    nc = _get_nc(fold)
    in_maps = prep_in_maps(x_hyp, W1, b1, W2, b2, W3, b3)
    res = bass_utils.run_bass_kernel_spmd(nc, in_maps,
                                          core_ids=list(range(N_CORES)))
    parts = [np.asarray(res.results[c]["out"]).T for c in range(N_CORES)]
    return np.ascontiguousarray(np.concatenate(parts, axis=0),
                                dtype=np.float32)
